# revision 5
# baseline (speedup 1.0000x reference)
"""Trainium2 Bass kernel for DiscreteSender (LSTM greedy decoder).

Data-parallel over 8 NeuronCores: batch 4096 -> 512 rows/core, weights
replicated. Per core everything is kept feature-major ("transposed") so the
LSTM recurrence needs no transposes:

  h^T, c^T        [H=512 part-tiles, b=512 free]
  gates^T         [4H=2048 part-tiles, b free]   (PE, fp32)
  logits          [b part-tiles, V=1024 free]    (PE, fp32; argmax on DVE)
  emb feedback    dma_gather rows of output_emb by argmax, PE-transpose

Full fp32 matmuls everywhere: the greedy argmax feeds back into the
recurrence, so bf16/fp32r-level logits error flips symbols vs the fp32
reference and wrecks the logits comparison downstream.
"""

import numpy as np

B, A, NE, V, L, DIN, DOUT, H = 4096, 6, 100, 1024, 32, 64, 64, 512
NCORES = 8
BC = B // NCORES           # 512 rows per core
P = 128
NBT = BC // P              # 4 batch tiles
NKH = H // P               # 4 contraction tiles over H
NM = 4 * H // P            # 16 gate feature tiles
G4 = 4 * H

_CACHE = {}


def _build_nc(steps=L):
    from contextlib import ExitStack

    import concourse.bass as bass
    import concourse.tile as tile
    from concourse import bacc, mybir
    from concourse.masks import make_identity

    f32 = mybir.dt.float32
    i16 = mybir.dt.int16
    i32 = mybir.dt.int32
    u16 = mybir.dt.uint16
    AF = mybir.ActivationFunctionType

    nc = bacc.Bacc("TRN2", target_bir_lowering=False, debug=False,
                   num_devices=NCORES)

    whhT = nc.declare_dram_parameter("whhT", [P, NKH, G4], f32, isOutput=False)
    wihT = nc.declare_dram_parameter("wihT", [DOUT, G4], f32, isOutput=False)
    wo = nc.declare_dram_parameter("wo", [P, NKH, V], f32, isOutput=False)
    wi = nc.declare_dram_parameter("wi", [P, 3, H], f32, isOutput=False)
    bias_g = nc.declare_dram_parameter("bias_g", [P, NM], f32, isOutput=False)
    bi_col = nc.declare_dram_parameter("bi_col", [P, NKH], f32, isOutput=False)
    bo_bc = nc.declare_dram_parameter("bo_bc", [P, V], f32, isOutput=False)
    sos_col = nc.declare_dram_parameter("sos_col", [DOUT, 1], f32, isOutput=False)
    oemb = nc.declare_dram_parameter("oemb", [V, DOUT], f32, isOutput=False)
    iemb = nc.declare_dram_parameter("iemb", [NE, DOUT], f32, isOutput=False)
    xind = nc.declare_dram_parameter("xind", [P, NBT * A], i32, isOutput=False)
    seq_out = nc.declare_dram_parameter("seq", [BC, steps], i32, isOutput=True)
    log_out = nc.declare_dram_parameter("logits", [BC, steps, V], f32, isOutput=True)

    with tile.TileContext(nc) as tc, ExitStack() as ctx:
        const = ctx.enter_context(tc.tile_pool(name="const", bufs=1))
        state = ctx.enter_context(tc.tile_pool(name="state", bufs=2))
        gwork = ctx.enter_context(tc.tile_pool(name="gwork", bufs=1))
        lwork = ctx.enter_context(tc.tile_pool(name="lwork", bufs=3))
        misc = ctx.enter_context(tc.tile_pool(name="misc", bufs=2))
        psumg = ctx.enter_context(tc.tile_pool(name="psumg", bufs=4, space="PSUM"))
        psuml = ctx.enter_context(tc.tile_pool(name="psuml", bufs=2, space="PSUM"))
        psumt = ctx.enter_context(tc.tile_pool(name="psumt", bufs=2, space="PSUM"))

        # ---- resident constants ----
        whh_sb = const.tile([P, NKH, G4], f32)
        nc.sync.dma_start(whh_sb[:], whhT[:, :, :])
        wih_sb = const.tile([DOUT, G4], f32)
        nc.sync.dma_start(wih_sb[:], wihT[:, :])
        wo_sb = const.tile([P, NKH, V], f32)
        nc.sync.dma_start(wo_sb[:], wo[:, :, :])
        wi_sb = const.tile([P, 3, H], f32)
        nc.sync.dma_start(wi_sb[:], wi[:, :, :])
        bg_sb = const.tile([P, NM], f32)
        nc.sync.dma_start(bg_sb[:], bias_g[:, :])
        bi_sb = const.tile([P, NKH], f32)
        nc.sync.dma_start(bi_sb[:], bi_col[:, :])
        bo_sb = const.tile([P, V], f32)
        nc.sync.dma_start(bo_sb[:], bo_bc[:, :])
        sos_sb = const.tile([DOUT, 1], f32)
        nc.sync.dma_start(sos_sb[:], sos_col[:, :])
        ident = const.tile([P, P], f32)
        make_identity(nc, ident[:])
        seqi = const.tile([P, NBT, L], i32)

        # ---- init: e = input_emb[x], h0^T = Wi^T e^T + bi, c0 = 0 ----
        xi_sb = const.tile([P, NBT * A], i32)
        nc.sync.dma_start(xi_sb[:], xind[:, :])
        egath = misc.tile([P, NBT * A, DOUT], f32, tag="egath")
        for j in range(NBT * A):
            nc.gpsimd.indirect_dma_start(
                out=egath[:, j, :], out_offset=None,
                in_=iemb[:, :],
                in_offset=bass.IndirectOffsetOnAxis(ap=xi_sb[:, j:j + 1], axis=0))
        eT = const.tile([P, 3, BC], f32)
        for t in range(NBT):
            for kk in range(3):
                pst = psumt.tile([P, P], f32, tag="tr")
                nc.tensor.transpose(
                    pst[:], egath[:, t * A + kk * 2: t * A + kk * 2 + 2, :],
                    ident[:])
                nc.scalar.copy(eT[:, kk, t * P:(t + 1) * P], pst[:, :])

        hT = []
        for m in range(NKH):
            ps = psumg.tile([P, BC], f32, tag="gps")
            for k in range(3):
                nc.tensor.matmul(ps[:], wi_sb[:, k, m * P:(m + 1) * P],
                                 eT[:, k, :], start=(k == 0), stop=(k == 2))
            h = state.tile([P, BC], f32, tag=f"hT{m}")
            nc.scalar.activation(h[:], ps[:], AF.Identity, bias=bi_sb[:, m:m + 1])
            hT.append(h)

        cT = []
        for j in range(NKH):
            c = state.tile([P, BC], f32, tag=f"cT{j}")
            nc.vector.memset(c[:], 0.0)
            cT.append(c)

        z64 = misc.tile([DOUT, BC], f32, tag="z64")
        nc.vector.memset(z64[:], 0.0)
        embT = state.tile([DOUT, BC], f32, tag="embT")
        nc.scalar.activation(embT[:], z64[:], AF.Identity, bias=sos_sb[:, 0:1])

        # ---- recurrence ----
        for t in range(steps):
            # gates^T = W_hh h^T + W_ih emb^T + b  -> i,f,g,o (ACT)
            gsb = []
            for m in range(NM):
                ps = psumg.tile([P, BC], f32, tag="gps")
                for k in range(NKH):
                    nc.tensor.matmul(ps[:], whh_sb[:, k, m * P:(m + 1) * P],
                                     hT[k][:], start=(k == 0), stop=False)
                nc.tensor.matmul(ps[:], wih_sb[:, m * P:(m + 1) * P],
                                 embT[:], start=False, stop=True)
                g = gwork.tile([P, BC], f32, tag=f"g{m}")
                func = AF.Tanh if m // NKH == 2 else AF.Sigmoid
                nc.scalar.activation(g[:], ps[:], func, bias=bg_sb[:, m:m + 1])
                gsb.append(g)

            # cell: c = f*c + i*g ; h = o * tanh(c)
            hT_new, cT_new = [], []
            for j in range(NKH):
                ig = misc.tile([P, BC], f32, tag="ig")
                nc.vector.tensor_mul(ig[:], gsb[j][:], gsb[2 * NKH + j][:])
                fc = misc.tile([P, BC], f32, tag="fc")
                nc.vector.tensor_mul(fc[:], gsb[NKH + j][:], cT[j][:])
                cn = state.tile([P, BC], f32, tag=f"cT{j}")
                nc.vector.tensor_add(cn[:], ig[:], fc[:])
                th = misc.tile([P, BC], f32, tag="th")
                nc.scalar.activation(th[:], cn[:], AF.Tanh)
                hn = state.tile([P, BC], f32, tag=f"hT{j}")
                nc.vector.tensor_mul(hn[:], gsb[3 * NKH + j][:], th[:])
                cT_new.append(cn)
                hT_new.append(hn)
            hT, cT = hT_new, cT_new

            # logits = h Wo + bo ; argmax -> sym ; emb = output_emb[sym]
            for bt in range(NBT):
                lsb = lwork.tile([P, V], f32, tag="lsb")
                for vb in range(2):
                    pl = psuml.tile([P, 512], f32, tag="lps")
                    for k in range(NKH):
                        nc.tensor.matmul(
                            pl[:], hT[k][:, bt * P:(bt + 1) * P],
                            wo_sb[:, k, vb * 512:(vb + 1) * 512],
                            start=(k == 0), stop=(k == NKH - 1))
                    nc.vector.tensor_add(lsb[:, vb * 512:(vb + 1) * 512],
                                         pl[:], bo_sb[:, vb * 512:(vb + 1) * 512])
                nc.sync.dma_start(
                    log_out[bt * P:(bt + 1) * P, t, :].unsqueeze(1),
                    lsb[:].unsqueeze(1))
                mx8 = misc.tile([P, 8], f32, tag="mx8")
                nc.vector.max(mx8[:], lsb[:])
                mi8 = misc.tile([P, 8], u16, tag="mi8")
                nc.vector.max_index(mi8[:], mx8[:], lsb[:])
                nc.vector.tensor_copy(seqi[:, bt, t:t + 1], mi8[:, 0:1])

            if t == steps - 1:
                break

            # emb = output_emb[sym]: indirect row gather + PE transpose
            egn = misc.tile([P, NBT, DOUT], f32, tag="egn")
            for bt in range(NBT):
                nc.gpsimd.indirect_dma_start(
                    out=egn[:, bt, :], out_offset=None,
                    in_=oemb[:, :],
                    in_offset=bass.IndirectOffsetOnAxis(
                        ap=seqi[:, bt, t:t + 1], axis=0))
            embT = state.tile([DOUT, BC], f32, tag="embT")
            for bt in range(NBT):
                pse = psumt.tile([P, P], f32, tag="tr")
                nc.tensor.transpose(pse[:DOUT, :], egn[:, bt, :], ident[:])
                nc.scalar.copy(embT[:, bt * P:(bt + 1) * P], pse[:DOUT, :])

        # ---- outputs ----
        for bt in range(NBT):
            nc.sync.dma_start(seq_out[bt * P:(bt + 1) * P, :],
                              seqi[:, bt, :steps])

    nc.finalize()
    return nc


def _prep_inputs(x, input_emb, output_emb, Wi, bi, W_ih, W_hh, b_ih, b_hh,
                 Wo, bo, sos):
    f = np.float32
    x = np.asarray(x)
    W_hh = np.asarray(W_hh, f)
    W_ih = np.asarray(W_ih, f)
    Wo_ = np.asarray(Wo, f)
    Wi_ = np.asarray(Wi, f)
    common = {
        "whhT": np.ascontiguousarray(
            W_hh.T.reshape(NKH, P, G4).transpose(1, 0, 2)),
        "wihT": np.ascontiguousarray(W_ih.T),
        "wo": np.ascontiguousarray(Wo_.reshape(NKH, P, V).transpose(1, 0, 2)),
        "wi": np.ascontiguousarray(Wi_.reshape(3, P, H).transpose(1, 0, 2)),
        "bias_g": np.ascontiguousarray(
            (np.asarray(b_ih, f) + np.asarray(b_hh, f)).reshape(NM, P).T),
        "bi_col": np.ascontiguousarray(np.asarray(bi, f).reshape(NKH, P).T),
        "bo_bc": np.ascontiguousarray(
            np.broadcast_to(np.asarray(bo, f), (P, V))),
        "sos_col": np.ascontiguousarray(np.asarray(sos, f).reshape(DOUT, 1)),
        "oemb": np.ascontiguousarray(np.asarray(output_emb, f)),
        "iemb": np.ascontiguousarray(np.asarray(input_emb, f)),
    }
    in_maps = []
    for c in range(NCORES):
        xc = x[c * BC:(c + 1) * BC].astype(np.int64)
        # xind[p, bt*A + a] = x[bt*128 + p, a]
        jj = np.arange(NBT * A)
        pp = np.arange(P)
        mat = xc[(jj[None, :] // A) * P + pp[:, None], jj[None, :] % A]
        m = dict(common)
        m["xind"] = np.ascontiguousarray(mat.astype(np.int32))
        in_maps.append(m)
    return in_maps


def _get_nc(steps=L):
    if steps not in _CACHE:
        _CACHE[steps] = _build_nc(steps)
    return _CACHE[steps]


def kernel(x, input_emb, output_emb, Wi, bi, W_ih, W_hh, b_ih, b_hh, Wo, bo,
           sos, _steps=L, _trace=False):
    from concourse.bass_utils import run_bass_kernel_spmd

    nc = _get_nc(_steps)
    in_maps = _prep_inputs(x, input_emb, output_emb, Wi, bi, W_ih, W_hh,
                           b_ih, b_hh, Wo, bo, sos)
    res = run_bass_kernel_spmd(nc, in_maps, list(range(NCORES)), trace=_trace)
    kernel.last_result = res
    seq = np.concatenate([res.results[c]["seq"] for c in range(NCORES)], axis=0)
    logits = np.concatenate([res.results[c]["logits"] for c in range(NCORES)],
                            axis=0)
    if np.asarray(x).dtype == np.int64:
        seq = seq.astype(np.int64)
    else:
        seq = seq.astype(np.int32)
    return seq, logits


# revision 7
# speedup vs baseline: 1.5627x; 1.5627x over previous
"""Trainium2 Bass kernel for DiscreteSender (LSTM greedy decoder).

Data-parallel over 8 NeuronCores: batch 4096 -> 512 rows/core, weights
replicated. Per core everything is kept feature-major ("transposed") so the
LSTM recurrence needs no transposes:

  h^T, c^T        [H=512 part-tiles, b=512 free]
  gates^T         [4H=2048 part-tiles, b free]   (PE)
  logits          [b part-tiles, V=1024 free]    (PE; argmax top-8 on DVE)
  emb feedback    indirect-DMA row gather of output_emb by argmax + PE transpose

Matmul precision: fp16 hi/lo x3 split (x@W ~= x_hi@W_hi + x_hi@W_lo +
x_lo@W_hi, all accumulated in fp32 PSUM). This matches fp32 accuracy
(~4e-7 measured, PE keeps fp16 subnormals) at fp16 streaming rate - the
greedy argmax feeds back into the recurrence, so anything at bf16/fp32r
precision flips symbols vs the fp32 reference and is unusable.
"""

import numpy as np

B, A, NE, V, L, DIN, DOUT, H = 4096, 6, 100, 1024, 32, 64, 64, 512
NCORES = 8
BC = B // NCORES           # 512 rows per core
P = 128
NBT = BC // P              # 4 batch tiles
NKH = H // P               # 4 contraction tiles over H
NM = 4 * H // P            # 16 gate feature tiles
G4 = 4 * H

_CACHE = {}


def _build_nc(steps=L):
    from contextlib import ExitStack

    import concourse.bass as bass
    import concourse.tile as tile
    from concourse import bacc, mybir
    from concourse.masks import make_identity

    f32 = mybir.dt.float32
    f16 = mybir.dt.float16
    i32 = mybir.dt.int32
    u16 = mybir.dt.uint16
    AF = mybir.ActivationFunctionType

    nc = bacc.Bacc("TRN2", target_bir_lowering=False, debug=False,
                   num_devices=NCORES)

    whh_hi = nc.declare_dram_parameter("whh_hi", [P, NKH, G4], f16, isOutput=False)
    whh_lo = nc.declare_dram_parameter("whh_lo", [P, NKH, G4], f16, isOutput=False)
    wih_hi = nc.declare_dram_parameter("wih_hi", [DOUT, G4], f16, isOutput=False)
    wih_aux = nc.declare_dram_parameter("wih_aux", [P, G4], f16, isOutput=False)
    wo_hi = nc.declare_dram_parameter("wo_hi", [P, NKH, V], f16, isOutput=False)
    wo_lo = nc.declare_dram_parameter("wo_lo", [P, NKH, V], f16, isOutput=False)
    wi = nc.declare_dram_parameter("wi", [P, 3, H], f32, isOutput=False)
    bias_g = nc.declare_dram_parameter("bias_g", [P, NM], f32, isOutput=False)
    bi_col = nc.declare_dram_parameter("bi_col", [P, NKH], f32, isOutput=False)
    bo_bc = nc.declare_dram_parameter("bo_bc", [P, V], f32, isOutput=False)
    sos_col = nc.declare_dram_parameter("sos_col", [DOUT, 1], f32, isOutput=False)
    oemb = nc.declare_dram_parameter("oemb", [V, DOUT], f32, isOutput=False)
    iemb = nc.declare_dram_parameter("iemb", [NE, DOUT], f32, isOutput=False)
    xind = nc.declare_dram_parameter("xind", [P, NBT * A], i32, isOutput=False)
    seq_out = nc.declare_dram_parameter("seq", [BC, steps], i32, isOutput=True)
    log_out = nc.declare_dram_parameter("logits", [BC, steps, V], f32, isOutput=True)

    with tile.TileContext(nc) as tc, ExitStack() as ctx:
        const = ctx.enter_context(tc.tile_pool(name="const", bufs=1))
        state = ctx.enter_context(tc.tile_pool(name="state", bufs=2))
        gwork = ctx.enter_context(tc.tile_pool(name="gwork", bufs=1))
        lwork = ctx.enter_context(tc.tile_pool(name="lwork", bufs=3))
        misc = ctx.enter_context(tc.tile_pool(name="misc", bufs=2))
        initp = ctx.enter_context(tc.tile_pool(name="initp", bufs=1))
        psumg = ctx.enter_context(tc.tile_pool(name="psumg", bufs=4, space="PSUM"))
        psuml = ctx.enter_context(tc.tile_pool(name="psuml", bufs=2, space="PSUM"))
        psumt = ctx.enter_context(tc.tile_pool(name="psumt", bufs=2, space="PSUM"))

        # ---- resident constants ----
        whhh_sb = const.tile([P, NKH, G4], f16)
        nc.sync.dma_start(whhh_sb[:], whh_hi[:, :, :])
        whhl_sb = const.tile([P, NKH, G4], f16)
        nc.sync.dma_start(whhl_sb[:], whh_lo[:, :, :])
        wihh_sb = const.tile([DOUT, G4], f16)
        nc.sync.dma_start(wihh_sb[:], wih_hi[:, :])
        wiha_sb = const.tile([P, G4], f16)
        nc.sync.dma_start(wiha_sb[:], wih_aux[:, :])
        woh_sb = const.tile([P, NKH, V], f16)
        nc.sync.dma_start(woh_sb[:], wo_hi[:, :, :])
        wol_sb = const.tile([P, NKH, V], f16)
        nc.sync.dma_start(wol_sb[:], wo_lo[:, :, :])
        wi_sb = const.tile([P, 3, H], f32)
        nc.sync.dma_start(wi_sb[:], wi[:, :, :])
        bg_sb = const.tile([P, NM], f32)
        nc.sync.dma_start(bg_sb[:], bias_g[:, :])
        bi_sb = const.tile([P, NKH], f32)
        nc.sync.dma_start(bi_sb[:], bi_col[:, :])
        bo_sb = const.tile([P, V], f32)
        nc.sync.dma_start(bo_sb[:], bo_bc[:, :])
        sos_sb = const.tile([DOUT, 1], f32)
        nc.sync.dma_start(sos_sb[:], sos_col[:, :])
        ident = const.tile([P, P], f32)
        make_identity(nc, ident[:])
        seqi = const.tile([P, NBT, L], i32)

        def split16(src_f32, dst_hi16, dst_lo16):
            # hi = f16(x) on ACT; lo = f16(x - hi) on DVE (mixed-dtype TT)
            nc.scalar.copy(dst_hi16, src_f32)
            nc.vector.tensor_tensor(out=dst_lo16, in0=src_f32, in1=dst_hi16,
                                    op=mybir.AluOpType.subtract)

        # ---- init: e = input_emb[x], h0^T = Wi^T e^T + bi, c0 = 0 ----
        xi_sb = const.tile([P, NBT * A], i32)
        nc.sync.dma_start(xi_sb[:], xind[:, :])
        egath = initp.tile([P, NBT * A, DOUT], f32, tag="egath")
        for j in range(NBT * A):
            nc.gpsimd.indirect_dma_start(
                out=egath[:, j, :], out_offset=None,
                in_=iemb[:, :],
                in_offset=bass.IndirectOffsetOnAxis(ap=xi_sb[:, j:j + 1], axis=0))
        eT = const.tile([P, 3, BC], f32)
        for t in range(NBT):
            for kk in range(3):
                pst = psumt.tile([P, P], f32, tag="tr")
                nc.tensor.transpose(
                    pst[:], egath[:, t * A + kk * 2: t * A + kk * 2 + 2, :],
                    ident[:])
                nc.scalar.copy(eT[:, kk, t * P:(t + 1) * P], pst[:, :])

        h_hi, h_lo = [], []
        for m in range(NKH):
            ps = psumg.tile([P, BC], f32, tag="gps")
            for k in range(3):
                nc.tensor.matmul(ps[:], wi_sb[:, k, m * P:(m + 1) * P],
                                 eT[:, k, :], start=(k == 0), stop=(k == 2))
            h = misc.tile([P, BC], f32, tag="hn")
            nc.scalar.activation(h[:], ps[:], AF.Identity, bias=bi_sb[:, m:m + 1])
            hh = state.tile([P, BC], f16, tag=f"hh{m}")
            hl = state.tile([P, BC], f16, tag=f"hl{m}")
            split16(h[:], hh[:], hl[:])
            h_hi.append(hh)
            h_lo.append(hl)

        cT = []
        for j in range(NKH):
            c = state.tile([P, BC], f32, tag=f"cT{j}")
            nc.vector.memset(c[:], 0.0)
            cT.append(c)

        z64 = initp.tile([DOUT, BC], f32, tag="z64")
        nc.vector.memset(z64[:], 0.0)
        embT = state.tile([DOUT, BC], f32, tag="embT")
        nc.scalar.activation(embT[:], z64[:], AF.Identity, bias=sos_sb[:, 0:1])
        embcat = state.tile([P, BC], f16, tag="embcat")
        split16(embT[:], embcat[0:DOUT, :], embcat[DOUT:P, :])

        # ---- recurrence ----
        for t in range(steps):
            # gates^T = W_hh h^T + W_ih emb^T + b  (fp16 x3)  -> i,f,g,o
            gsb = []
            for m in range(NM):
                ms = slice(m * P, (m + 1) * P)
                ps = psumg.tile([P, BC], f32, tag="gps")
                nc.tensor.matmul(ps[:], whhh_sb[:, 0, ms], h_hi[0][:],
                                 start=True, stop=False)
                for k in range(1, NKH):
                    nc.tensor.matmul(ps[:], whhh_sb[:, k, ms], h_hi[k][:],
                                     start=False, stop=False)
                for k in range(NKH):
                    nc.tensor.matmul(ps[:], whhl_sb[:, k, ms], h_hi[k][:],
                                     start=False, stop=False)
                for k in range(NKH):
                    nc.tensor.matmul(ps[:], whhh_sb[:, k, ms], h_lo[k][:],
                                     start=False, stop=False)
                nc.tensor.matmul(ps[:], wihh_sb[:, ms], embcat[0:DOUT, :],
                                 start=False, stop=False)
                nc.tensor.matmul(ps[:], wiha_sb[:, ms], embcat[:],
                                 start=False, stop=True)
                g = gwork.tile([P, BC], f32, tag=f"g{m}")
                func = AF.Tanh if m // NKH == 2 else AF.Sigmoid
                nc.scalar.activation(g[:], ps[:], func, bias=bg_sb[:, m:m + 1])
                gsb.append(g)

            # cell: c = f*c + i*g ; h = o * tanh(c) ; split h -> (hi, lo)
            cT_new, hhi_new, hlo_new = [], [], []
            for j in range(NKH):
                ig = misc.tile([P, BC], f32, tag="ig")
                nc.vector.tensor_mul(ig[:], gsb[j][:], gsb[2 * NKH + j][:])
                fc = misc.tile([P, BC], f32, tag="fc")
                nc.vector.tensor_mul(fc[:], gsb[NKH + j][:], cT[j][:])
                cn = state.tile([P, BC], f32, tag=f"cT{j}")
                nc.vector.tensor_add(cn[:], ig[:], fc[:])
                th = misc.tile([P, BC], f32, tag="th")
                nc.scalar.activation(th[:], cn[:], AF.Tanh)
                hn = misc.tile([P, BC], f32, tag="hn")
                nc.vector.tensor_mul(hn[:], gsb[3 * NKH + j][:], th[:])
                hh = state.tile([P, BC], f16, tag=f"hh{j}")
                hl = state.tile([P, BC], f16, tag=f"hl{j}")
                split16(hn[:], hh[:], hl[:])
                cT_new.append(cn)
                hhi_new.append(hh)
                hlo_new.append(hl)
            cT, h_hi, h_lo = cT_new, hhi_new, hlo_new

            # logits = h Wo + bo ; argmax -> sym
            for bt in range(NBT):
                bs = slice(bt * P, (bt + 1) * P)
                lsb = lwork.tile([P, V], f32, tag="lsb")
                for vb in range(2):
                    vs = slice(vb * 512, (vb + 1) * 512)
                    pl = psuml.tile([P, 512], f32, tag="lps")
                    for k in range(NKH):
                        nc.tensor.matmul(pl[:], h_hi[k][:, bs], woh_sb[:, k, vs],
                                         start=(k == 0), stop=False)
                    for k in range(NKH):
                        nc.tensor.matmul(pl[:], h_hi[k][:, bs], wol_sb[:, k, vs],
                                         start=False, stop=False)
                    for k in range(NKH):
                        nc.tensor.matmul(pl[:], h_lo[k][:, bs], woh_sb[:, k, vs],
                                         start=False, stop=(k == NKH - 1))
                    nc.vector.tensor_add(lsb[:, vs], pl[:], bo_sb[:, vs])
                nc.sync.dma_start(
                    log_out[bt * P:(bt + 1) * P, t, :].unsqueeze(1),
                    lsb[:].unsqueeze(1))
                mx8 = misc.tile([P, 8], f32, tag="mx8")
                nc.vector.max(mx8[:], lsb[:])
                mi8 = misc.tile([P, 8], u16, tag="mi8")
                nc.vector.max_index(mi8[:], mx8[:], lsb[:])
                nc.vector.tensor_copy(seqi[:, bt, t:t + 1], mi8[:, 0:1])

            if t == steps - 1:
                break

            # emb = output_emb[sym]: indirect row gather + PE transpose + split
            egn = misc.tile([P, NBT, DOUT], f32, tag="egn")
            for bt in range(NBT):
                nc.gpsimd.indirect_dma_start(
                    out=egn[:, bt, :], out_offset=None,
                    in_=oemb[:, :],
                    in_offset=bass.IndirectOffsetOnAxis(
                        ap=seqi[:, bt, t:t + 1], axis=0))
            embT = state.tile([DOUT, BC], f32, tag="embT")
            for bt in range(NBT):
                pse = psumt.tile([P, P], f32, tag="tr")
                nc.tensor.transpose(pse[:DOUT, :], egn[:, bt, :], ident[:])
                nc.scalar.copy(embT[:, bt * P:(bt + 1) * P], pse[:DOUT, :])
            embcat = state.tile([P, BC], f16, tag="embcat")
            split16(embT[:], embcat[0:DOUT, :], embcat[DOUT:P, :])

        # ---- outputs ----
        for bt in range(NBT):
            nc.sync.dma_start(seq_out[bt * P:(bt + 1) * P, :],
                              seqi[:, bt, :steps])

    nc.finalize()
    return nc


def _split16_np(x):
    hi = x.astype(np.float16)
    lo = (x - hi.astype(np.float32)).astype(np.float16)
    return hi, lo


def _prep_inputs(x, input_emb, output_emb, Wi, bi, W_ih, W_hh, b_ih, b_hh,
                 Wo, bo, sos):
    f = np.float32
    x = np.asarray(x)
    W_hh = np.asarray(W_hh, f)
    W_ih = np.asarray(W_ih, f)
    Wo_ = np.asarray(Wo, f)
    Wi_ = np.asarray(Wi, f)

    whhT = np.ascontiguousarray(W_hh.T.reshape(NKH, P, G4).transpose(1, 0, 2))
    whh_hi, whh_lo = _split16_np(whhT)
    wo_t = np.ascontiguousarray(Wo_.reshape(NKH, P, V).transpose(1, 0, 2))
    wo_hi, wo_lo = _split16_np(wo_t)
    wihT = np.ascontiguousarray(W_ih.T)                    # [64, 2048]
    wih_hi, wih_lo = _split16_np(wihT)

    common = {
        "whh_hi": whh_hi,
        "whh_lo": whh_lo,
        "wih_hi": wih_hi,
        "wih_aux": np.ascontiguousarray(np.concatenate([wih_lo, wih_hi], axis=0)),
        "wo_hi": wo_hi,
        "wo_lo": wo_lo,
        "wi": np.ascontiguousarray(Wi_.reshape(3, P, H).transpose(1, 0, 2)),
        "bias_g": np.ascontiguousarray(
            (np.asarray(b_ih, f) + np.asarray(b_hh, f)).reshape(NM, P).T),
        "bi_col": np.ascontiguousarray(np.asarray(bi, f).reshape(NKH, P).T),
        "bo_bc": np.ascontiguousarray(
            np.broadcast_to(np.asarray(bo, f), (P, V))),
        "sos_col": np.ascontiguousarray(np.asarray(sos, f).reshape(DOUT, 1)),
        "oemb": np.ascontiguousarray(np.asarray(output_emb, f)),
        "iemb": np.ascontiguousarray(np.asarray(input_emb, f)),
    }
    in_maps = []
    for c in range(NCORES):
        xc = x[c * BC:(c + 1) * BC].astype(np.int64)
        # xind[p, bt*A + a] = x[bt*128 + p, a]
        jj = np.arange(NBT * A)
        pp = np.arange(P)
        mat = xc[(jj[None, :] // A) * P + pp[:, None], jj[None, :] % A]
        m = dict(common)
        m["xind"] = np.ascontiguousarray(mat.astype(np.int32))
        in_maps.append(m)
    return in_maps


def _get_nc(steps=L):
    if steps not in _CACHE:
        _CACHE[steps] = _build_nc(steps)
    return _CACHE[steps]


def kernel(x, input_emb, output_emb, Wi, bi, W_ih, W_hh, b_ih, b_hh, Wo, bo,
           sos, _steps=L, _trace=False):
    from concourse.bass_utils import run_bass_kernel_spmd

    nc = _get_nc(_steps)
    in_maps = _prep_inputs(x, input_emb, output_emb, Wi, bi, W_ih, W_hh,
                           b_ih, b_hh, Wo, bo, sos)
    res = run_bass_kernel_spmd(nc, in_maps, list(range(NCORES)), trace=_trace)
    kernel.last_result = res
    seq = np.concatenate([res.results[c]["seq"] for c in range(NCORES)], axis=0)
    logits = np.concatenate([res.results[c]["logits"] for c in range(NCORES)],
                            axis=0)
    if np.asarray(x).dtype == np.int64:
        seq = seq.astype(np.int64)
    else:
        seq = seq.astype(np.int32)
    return seq, logits


# revision 8
# speedup vs baseline: 1.5630x; 1.0002x over previous
"""Trainium2 Bass kernel for DiscreteSender (LSTM greedy decoder).

Data-parallel over 8 NeuronCores: batch 4096 -> 512 rows/core, weights
replicated. Per core everything is kept feature-major ("transposed") so the
LSTM recurrence needs no transposes:

  h^T, c^T        [H=512 part-tiles, b=512 free]
  gates^T         [4H=2048 part-tiles, b free]   (PE)
  logits          [b part-tiles, V=1024 free]    (PE; argmax top-8 on DVE)
  emb feedback    indirect-DMA row gather of output_emb by argmax + PE transpose

Matmul precision: fp16 hi/lo x3 split (x@W ~= x_hi@W_hi + x_hi@W_lo +
x_lo@W_hi, all accumulated in fp32 PSUM). This matches fp32 accuracy
(~4e-7 measured, PE keeps fp16 subnormals) at fp16 streaming rate - the
greedy argmax feeds back into the recurrence, so anything at bf16/fp32r
precision flips symbols vs the fp32 reference and is unusable.
"""

import numpy as np

B, A, NE, V, L, DIN, DOUT, H = 4096, 6, 100, 1024, 32, 64, 64, 512
NCORES = 8
BC = B // NCORES           # 512 rows per core
P = 128
NBT = BC // P              # 4 batch tiles
NKH = H // P               # 4 contraction tiles over H
NM = 4 * H // P            # 16 gate feature tiles
G4 = 4 * H

_CACHE = {}


def _build_nc(steps=L):
    from contextlib import ExitStack

    import concourse.bass as bass
    import concourse.tile as tile
    from concourse import bacc, mybir
    from concourse.masks import make_identity

    f32 = mybir.dt.float32
    f16 = mybir.dt.float16
    i32 = mybir.dt.int32
    u16 = mybir.dt.uint16
    AF = mybir.ActivationFunctionType

    nc = bacc.Bacc("TRN2", target_bir_lowering=False, debug=False,
                   num_devices=NCORES)

    whh_hi = nc.declare_dram_parameter("whh_hi", [P, NKH, G4], f16, isOutput=False)
    whh_lo = nc.declare_dram_parameter("whh_lo", [P, NKH, G4], f16, isOutput=False)
    wih_hi = nc.declare_dram_parameter("wih_hi", [DOUT, G4], f16, isOutput=False)
    wih_aux = nc.declare_dram_parameter("wih_aux", [P, G4], f16, isOutput=False)
    wo_hi = nc.declare_dram_parameter("wo_hi", [P, NKH, V], f16, isOutput=False)
    wo_lo = nc.declare_dram_parameter("wo_lo", [P, NKH, V], f16, isOutput=False)
    wi = nc.declare_dram_parameter("wi", [P, 3, H], f32, isOutput=False)
    bias_g = nc.declare_dram_parameter("bias_g", [P, NM], f32, isOutput=False)
    bi_col = nc.declare_dram_parameter("bi_col", [P, NKH], f32, isOutput=False)
    bo_bc = nc.declare_dram_parameter("bo_bc", [P, V], f32, isOutput=False)
    sos_col = nc.declare_dram_parameter("sos_col", [DOUT, 1], f32, isOutput=False)
    oemb = nc.declare_dram_parameter("oemb", [V, DOUT], f32, isOutput=False)
    iemb = nc.declare_dram_parameter("iemb", [NE, DOUT], f32, isOutput=False)
    xind = nc.declare_dram_parameter("xind", [P, NBT * A], i32, isOutput=False)
    seq_out = nc.declare_dram_parameter("seq", [BC, steps], i32, isOutput=True)
    log_out = nc.declare_dram_parameter("logits", [BC, steps, V], f32, isOutput=True)

    with tile.TileContext(nc) as tc, ExitStack() as ctx:
        const = ctx.enter_context(tc.tile_pool(name="const", bufs=1))
        state = ctx.enter_context(tc.tile_pool(name="state", bufs=2))
        gwork = ctx.enter_context(tc.tile_pool(name="gwork", bufs=1))
        lwork = ctx.enter_context(tc.tile_pool(name="lwork", bufs=3))
        misc = ctx.enter_context(tc.tile_pool(name="misc", bufs=2))
        initp = ctx.enter_context(tc.tile_pool(name="initp", bufs=1))
        psumg = ctx.enter_context(tc.tile_pool(name="psumg", bufs=4, space="PSUM"))
        psuml = ctx.enter_context(tc.tile_pool(name="psuml", bufs=3, space="PSUM"))
        psumt = ctx.enter_context(tc.tile_pool(name="psumt", bufs=1, space="PSUM"))

        # ---- resident constants ----
        whhh_sb = const.tile([P, NKH, G4], f16)
        nc.sync.dma_start(whhh_sb[:], whh_hi[:, :, :])
        whhl_sb = const.tile([P, NKH, G4], f16)
        nc.sync.dma_start(whhl_sb[:], whh_lo[:, :, :])
        wihh_sb = const.tile([DOUT, G4], f16)
        nc.sync.dma_start(wihh_sb[:], wih_hi[:, :])
        wiha_sb = const.tile([P, G4], f16)
        nc.sync.dma_start(wiha_sb[:], wih_aux[:, :])
        wi_sb = const.tile([P, 3, H], f32)
        nc.sync.dma_start(wi_sb[:], wi[:, :, :])
        bg_sb = const.tile([P, NM], f32)
        nc.sync.dma_start(bg_sb[:], bias_g[:, :])
        bi_sb = const.tile([P, NKH], f32)
        nc.sync.dma_start(bi_sb[:], bi_col[:, :])
        bo_sb = const.tile([P, V], f32)
        nc.sync.dma_start(bo_sb[:], bo_bc[:, :])
        sos_sb = const.tile([DOUT, 1], f32)
        nc.sync.dma_start(sos_sb[:], sos_col[:, :])
        woh_sb = const.tile([P, NKH, V], f16)
        nc.sync.dma_start(woh_sb[:], wo_hi[:, :, :])
        wol_sb = const.tile([P, NKH, V], f16)
        nc.sync.dma_start(wol_sb[:], wo_lo[:, :, :])
        ident = const.tile([P, P], f32)
        make_identity(nc, ident[:])
        seqi = const.tile([P, NBT, L], i32)

        def split16(src_f32, dst_hi16, dst_lo16):
            # hi = f16(x) on ACT; lo = f16(x - hi) on DVE (mixed-dtype TT)
            nc.scalar.copy(dst_hi16, src_f32)
            nc.vector.tensor_tensor(out=dst_lo16, in0=src_f32, in1=dst_hi16,
                                    op=mybir.AluOpType.subtract)

        # ---- init: e = input_emb[x], h0^T = Wi^T e^T + bi, c0 = 0 ----
        xi_sb = const.tile([P, NBT * A], i32)
        nc.sync.dma_start(xi_sb[:], xind[:, :])
        egath = initp.tile([P, NBT * A, DOUT], f32, tag="egath")
        for j in range(NBT * A):
            nc.gpsimd.indirect_dma_start(
                out=egath[:, j, :], out_offset=None,
                in_=iemb[:, :],
                in_offset=bass.IndirectOffsetOnAxis(ap=xi_sb[:, j:j + 1], axis=0))
        eT = const.tile([P, 3, BC], f32)
        for t in range(NBT):
            for kk in range(3):
                pst = psumt.tile([P, P], f32, tag="tr")
                nc.tensor.transpose(
                    pst[:], egath[:, t * A + kk * 2: t * A + kk * 2 + 2, :],
                    ident[:])
                nc.scalar.copy(eT[:, kk, t * P:(t + 1) * P], pst[:, :])

        h_hi, h_lo = [], []
        for m in range(NKH):
            ps = psumg.tile([P, BC], f32, tag="gps")
            for k in range(3):
                nc.tensor.matmul(ps[:], wi_sb[:, k, m * P:(m + 1) * P],
                                 eT[:, k, :], start=(k == 0), stop=(k == 2))
            h = misc.tile([P, BC], f32, tag="hn")
            nc.scalar.activation(h[:], ps[:], AF.Identity, bias=bi_sb[:, m:m + 1])
            hh = state.tile([P, BC], f16, tag=f"hh{m}")
            hl = state.tile([P, BC], f16, tag=f"hl{m}")
            split16(h[:], hh[:], hl[:])
            h_hi.append(hh)
            h_lo.append(hl)

        cT = []
        for j in range(NKH):
            c = state.tile([P, BC], f32, tag=f"cT{j}")
            nc.vector.memset(c[:], 0.0)
            cT.append(c)

        z64 = initp.tile([DOUT, BC], f32, tag="z64")
        nc.vector.memset(z64[:], 0.0)
        embT = state.tile([DOUT, BC], f32, tag="embT")
        nc.scalar.activation(embT[:], z64[:], AF.Identity, bias=sos_sb[:, 0:1])
        embcat = state.tile([P, BC], f16, tag="embcat")
        split16(embT[:], embcat[0:DOUT, :], embcat[DOUT:P, :])

        # ---- recurrence ----
        for t in range(steps):
            # gates^T = W_hh h^T + W_ih emb^T + b  (fp16 x3)  -> i,f,g,o
            # j-interleaved M order: after the 4 gate tiles of cell-slice j
            # are emitted, the cell update for j runs while PE continues
            # with slice j+1 (hides the ACT/DVE tail under matmuls).
            cT_new, hhi_new, hlo_new = [], [], []
            for j in range(NKH):
                gj = {}
                for m in (j, NKH + j, 2 * NKH + j, 3 * NKH + j):
                    ms = slice(m * P, (m + 1) * P)
                    ps = psumg.tile([P, BC], f32, tag="gps")
                    nc.tensor.matmul(ps[:], whhh_sb[:, 0, ms], h_hi[0][:],
                                     start=True, stop=False)
                    for k in range(1, NKH):
                        nc.tensor.matmul(ps[:], whhh_sb[:, k, ms], h_hi[k][:],
                                         start=False, stop=False)
                    for k in range(NKH):
                        nc.tensor.matmul(ps[:], whhl_sb[:, k, ms], h_hi[k][:],
                                         start=False, stop=False)
                    for k in range(NKH):
                        nc.tensor.matmul(ps[:], whhh_sb[:, k, ms], h_lo[k][:],
                                         start=False, stop=False)
                    nc.tensor.matmul(ps[:], wihh_sb[:, ms], embcat[0:DOUT, :],
                                     start=False, stop=False)
                    nc.tensor.matmul(ps[:], wiha_sb[:, ms], embcat[:],
                                     start=False, stop=True)
                    g = gwork.tile([P, BC], f32, tag=f"g{m}")
                    func = AF.Tanh if m // NKH == 2 else AF.Sigmoid
                    nc.scalar.activation(g[:], ps[:], func, bias=bg_sb[:, m:m + 1])
                    gj[m] = g
                # cell slice j: c = f*c + i*g ; h = o * tanh(c) ; split h
                ig = misc.tile([P, BC], f32, tag="ig")
                nc.vector.tensor_mul(ig[:], gj[j][:], gj[2 * NKH + j][:])
                fc = misc.tile([P, BC], f32, tag="fc")
                nc.vector.tensor_mul(fc[:], gj[NKH + j][:], cT[j][:])
                cn = state.tile([P, BC], f32, tag=f"cT{j}")
                nc.vector.tensor_add(cn[:], ig[:], fc[:])
                th = misc.tile([P, BC], f32, tag="th")
                nc.scalar.activation(th[:], cn[:], AF.Tanh)
                hn = misc.tile([P, BC], f32, tag="hn")
                nc.vector.tensor_mul(hn[:], gj[3 * NKH + j][:], th[:])
                hh = state.tile([P, BC], f16, tag=f"hh{j}")
                hl = state.tile([P, BC], f16, tag=f"hl{j}")
                split16(hn[:], hh[:], hl[:])
                cT_new.append(cn)
                hhi_new.append(hh)
                hlo_new.append(hl)
            cT, h_hi, h_lo = cT_new, hhi_new, hlo_new

            # logits = h Wo + bo ; argmax -> sym
            for bt in range(NBT):
                bs = slice(bt * P, (bt + 1) * P)
                lsb = lwork.tile([P, V], f32, tag="lsb")
                for vb in range(2):
                    vs = slice(vb * 512, (vb + 1) * 512)
                    pl = psuml.tile([P, 512], f32, tag="lps")
                    for k in range(NKH):
                        nc.tensor.matmul(pl[:], h_hi[k][:, bs], woh_sb[:, k, vs],
                                         start=(k == 0), stop=False)
                        nc.tensor.matmul(pl[:], h_hi[k][:, bs], wol_sb[:, k, vs],
                                         start=False, stop=False)
                        nc.tensor.matmul(pl[:], h_lo[k][:, bs], woh_sb[:, k, vs],
                                         start=False, stop=(k == NKH - 1))
                    nc.vector.tensor_add(lsb[:, vs], pl[:], bo_sb[:, vs])
                nc.sync.dma_start(
                    log_out[bt * P:(bt + 1) * P, t, :].unsqueeze(1),
                    lsb[:].unsqueeze(1))
                mx8 = misc.tile([P, 8], f32, tag="mx8")
                nc.vector.max(mx8[:], lsb[:])
                mi8 = misc.tile([P, 8], u16, tag="mi8")
                nc.vector.max_index(mi8[:], mx8[:], lsb[:])
                nc.vector.tensor_copy(seqi[:, bt, t:t + 1], mi8[:, 0:1])

            if t == steps - 1:
                break

            # emb = output_emb[sym]: indirect row gather + PE transpose + split
            egn = misc.tile([P, NBT, DOUT], f32, tag="egn")
            for bt in range(NBT):
                nc.gpsimd.indirect_dma_start(
                    out=egn[:, bt, :], out_offset=None,
                    in_=oemb[:, :],
                    in_offset=bass.IndirectOffsetOnAxis(
                        ap=seqi[:, bt, t:t + 1], axis=0))
            embT = state.tile([DOUT, BC], f32, tag="embT")
            for bt in range(NBT):
                pse = psumt.tile([P, P], f32, tag="tr")
                nc.tensor.transpose(pse[:DOUT, :], egn[:, bt, :], ident[:])
                nc.scalar.copy(embT[:, bt * P:(bt + 1) * P], pse[:DOUT, :])
            embcat = state.tile([P, BC], f16, tag="embcat")
            split16(embT[:], embcat[0:DOUT, :], embcat[DOUT:P, :])

        # ---- outputs ----
        for bt in range(NBT):
            nc.sync.dma_start(seq_out[bt * P:(bt + 1) * P, :],
                              seqi[:, bt, :steps])

    nc.finalize()
    return nc


def _split16_np(x):
    hi = x.astype(np.float16)
    lo = (x - hi.astype(np.float32)).astype(np.float16)
    return hi, lo


def _prep_inputs(x, input_emb, output_emb, Wi, bi, W_ih, W_hh, b_ih, b_hh,
                 Wo, bo, sos):
    f = np.float32
    x = np.asarray(x)
    W_hh = np.asarray(W_hh, f)
    W_ih = np.asarray(W_ih, f)
    Wo_ = np.asarray(Wo, f)
    Wi_ = np.asarray(Wi, f)

    whhT = np.ascontiguousarray(W_hh.T.reshape(NKH, P, G4).transpose(1, 0, 2))
    whh_hi, whh_lo = _split16_np(whhT)
    wo_t = np.ascontiguousarray(Wo_.reshape(NKH, P, V).transpose(1, 0, 2))
    wo_hi, wo_lo = _split16_np(wo_t)
    wihT = np.ascontiguousarray(W_ih.T)                    # [64, 2048]
    wih_hi, wih_lo = _split16_np(wihT)

    common = {
        "whh_hi": whh_hi,
        "whh_lo": whh_lo,
        "wih_hi": wih_hi,
        "wih_aux": np.ascontiguousarray(np.concatenate([wih_lo, wih_hi], axis=0)),
        "wo_hi": wo_hi,
        "wo_lo": wo_lo,
        "wi": np.ascontiguousarray(Wi_.reshape(3, P, H).transpose(1, 0, 2)),
        "bias_g": np.ascontiguousarray(
            (np.asarray(b_ih, f) + np.asarray(b_hh, f)).reshape(NM, P).T),
        "bi_col": np.ascontiguousarray(np.asarray(bi, f).reshape(NKH, P).T),
        "bo_bc": np.ascontiguousarray(
            np.broadcast_to(np.asarray(bo, f), (P, V))),
        "sos_col": np.ascontiguousarray(np.asarray(sos, f).reshape(DOUT, 1)),
        "oemb": np.ascontiguousarray(np.asarray(output_emb, f)),
        "iemb": np.ascontiguousarray(np.asarray(input_emb, f)),
    }
    in_maps = []
    for c in range(NCORES):
        xc = x[c * BC:(c + 1) * BC].astype(np.int64)
        # xind[p, bt*A + a] = x[bt*128 + p, a]
        jj = np.arange(NBT * A)
        pp = np.arange(P)
        mat = xc[(jj[None, :] // A) * P + pp[:, None], jj[None, :] % A]
        m = dict(common)
        m["xind"] = np.ascontiguousarray(mat.astype(np.int32))
        in_maps.append(m)
    return in_maps


def _get_nc(steps=L):
    if steps not in _CACHE:
        _CACHE[steps] = _build_nc(steps)
    return _CACHE[steps]


def kernel(x, input_emb, output_emb, Wi, bi, W_ih, W_hh, b_ih, b_hh, Wo, bo,
           sos, _steps=L, _trace=False):
    from concourse.bass_utils import run_bass_kernel_spmd

    nc = _get_nc(_steps)
    in_maps = _prep_inputs(x, input_emb, output_emb, Wi, bi, W_ih, W_hh,
                           b_ih, b_hh, Wo, bo, sos)
    res = run_bass_kernel_spmd(nc, in_maps, list(range(NCORES)), trace=_trace)
    kernel.last_result = res
    seq = np.concatenate([res.results[c]["seq"] for c in range(NCORES)], axis=0)
    logits = np.concatenate([res.results[c]["logits"] for c in range(NCORES)],
                            axis=0)
    if np.asarray(x).dtype == np.int64:
        seq = seq.astype(np.int64)
    else:
        seq = seq.astype(np.int32)
    return seq, logits


# revision 9
# speedup vs baseline: 1.5713x; 1.0053x over previous
"""Trainium2 Bass kernel for DiscreteSender (LSTM greedy decoder).

Data-parallel over 8 NeuronCores: batch 4096 -> 512 rows/core, weights
replicated. Per core everything is kept feature-major ("transposed") so the
LSTM recurrence needs no transposes:

  h^T, c^T        [H=512 part-tiles, b=512 free]
  gates^T         [4H=2048 part-tiles, b free]   (PE)
  logits          [b part-tiles, V=1024 free]    (PE; argmax top-8 on DVE)
  emb feedback    indirect-DMA row gather of output_emb by argmax + PE transpose

Matmul precision: fp16 hi/lo x3 split (x@W ~= x_hi@W_hi + x_hi@W_lo +
x_lo@W_hi, all accumulated in fp32 PSUM). This matches fp32 accuracy
(~4e-7 measured, PE keeps fp16 subnormals) at fp16 streaming rate - the
greedy argmax feeds back into the recurrence, so anything at bf16/fp32r
precision flips symbols vs the fp32 reference and is unusable.
"""

import numpy as np

B, A, NE, V, L, DIN, DOUT, H = 4096, 6, 100, 1024, 32, 64, 64, 512
NCORES = 8
BC = B // NCORES           # 512 rows per core
P = 128
NBT = BC // P              # 4 batch tiles
NKH = H // P               # 4 contraction tiles over H
NM = 4 * H // P            # 16 gate feature tiles
G4 = 4 * H

_CACHE = {}


def _build_nc(steps=L):
    from contextlib import ExitStack

    import concourse.bass as bass
    import concourse.tile as tile
    from concourse import bacc, mybir
    from concourse.masks import make_identity

    f32 = mybir.dt.float32
    f16 = mybir.dt.float16
    i32 = mybir.dt.int32
    u16 = mybir.dt.uint16
    AF = mybir.ActivationFunctionType

    nc = bacc.Bacc("TRN2", target_bir_lowering=False, debug=False,
                   num_devices=NCORES)

    whh_hi = nc.declare_dram_parameter("whh_hi", [P, NKH, G4], f16, isOutput=False)
    whh_lo = nc.declare_dram_parameter("whh_lo", [P, NKH, G4], f16, isOutput=False)
    wih_hi = nc.declare_dram_parameter("wih_hi", [DOUT, G4], f16, isOutput=False)
    wih_aux = nc.declare_dram_parameter("wih_aux", [P, G4], f16, isOutput=False)
    wo_hi = nc.declare_dram_parameter("wo_hi", [P, NKH, V], f16, isOutput=False)
    wo_lo = nc.declare_dram_parameter("wo_lo", [P, NKH, V], f16, isOutput=False)
    wi = nc.declare_dram_parameter("wi", [P, 3, H], f32, isOutput=False)
    bias_g = nc.declare_dram_parameter("bias_g", [P, NM], f32, isOutput=False)
    bi_col = nc.declare_dram_parameter("bi_col", [P, NKH], f32, isOutput=False)
    bo_bc = nc.declare_dram_parameter("bo_bc", [P, V], f32, isOutput=False)
    sos_col = nc.declare_dram_parameter("sos_col", [DOUT, 1], f32, isOutput=False)
    oemb = nc.declare_dram_parameter("oemb", [V, DOUT], f32, isOutput=False)
    iemb = nc.declare_dram_parameter("iemb", [NE, DOUT], f32, isOutput=False)
    xind = nc.declare_dram_parameter("xind", [P, NBT * A], i32, isOutput=False)
    seq_out = nc.declare_dram_parameter("seq", [BC, steps], i32, isOutput=True)
    log_out = nc.declare_dram_parameter("logits", [BC, steps, V], f32, isOutput=True)

    with tile.TileContext(nc) as tc, ExitStack() as ctx:
        const = ctx.enter_context(tc.tile_pool(name="const", bufs=1))
        state = ctx.enter_context(tc.tile_pool(name="state", bufs=2))
        gwork = ctx.enter_context(tc.tile_pool(name="gwork", bufs=1))
        lwork = ctx.enter_context(tc.tile_pool(name="lwork", bufs=3))
        misc = ctx.enter_context(tc.tile_pool(name="misc", bufs=2))
        initp = ctx.enter_context(tc.tile_pool(name="initp", bufs=1))
        psumg = ctx.enter_context(tc.tile_pool(name="psumg", bufs=5, space="PSUM"))
        psuml = ctx.enter_context(tc.tile_pool(name="psuml", bufs=2, space="PSUM"))
        psumt = ctx.enter_context(tc.tile_pool(name="psumt", bufs=1, space="PSUM"))

        # ---- resident constants ----
        whhh_sb = const.tile([P, NKH, G4], f16)
        for k in range(NKH):
            nc.sync.dma_start(whhh_sb[:, k, :], whh_hi[:, k, :])
        whhl_sb = const.tile([P, NKH, G4], f16)
        for k in range(NKH):
            nc.sync.dma_start(whhl_sb[:, k, :], whh_lo[:, k, :])
        wihh_sb = const.tile([DOUT, G4], f16)
        nc.sync.dma_start(wihh_sb[:], wih_hi[:, :])
        wiha_sb = const.tile([P, G4], f16)
        nc.sync.dma_start(wiha_sb[:], wih_aux[:, :])
        wi_sb = const.tile([P, 3, H], f32)
        nc.sync.dma_start(wi_sb[:], wi[:, :, :])
        bg_sb = const.tile([P, NM], f32)
        nc.sync.dma_start(bg_sb[:], bias_g[:, :])
        bi_sb = const.tile([P, NKH], f32)
        nc.sync.dma_start(bi_sb[:], bi_col[:, :])
        bo_sb = const.tile([P, V], f32)
        nc.sync.dma_start(bo_sb[:], bo_bc[:, :])
        sos_sb = const.tile([DOUT, 1], f32)
        nc.sync.dma_start(sos_sb[:], sos_col[:, :])
        woh_sb = const.tile([P, NKH, V], f16)
        nc.sync.dma_start(woh_sb[:], wo_hi[:, :, :])
        wol_sb = const.tile([P, NKH, V], f16)
        nc.sync.dma_start(wol_sb[:], wo_lo[:, :, :])
        ident = const.tile([P, P], f32)
        make_identity(nc, ident[:])
        seqi = const.tile([P, NBT, L], i32)

        def split16(src_f32, dst_hi16, dst_lo16):
            # hi = f16(x) on ACT; lo = f16(x - hi) on DVE (mixed-dtype TT)
            nc.scalar.copy(dst_hi16, src_f32)
            nc.vector.tensor_tensor(out=dst_lo16, in0=src_f32, in1=dst_hi16,
                                    op=mybir.AluOpType.subtract)

        # ---- init: e = input_emb[x], h0^T = Wi^T e^T + bi, c0 = 0 ----
        xi_sb = const.tile([P, NBT * A], i32)
        nc.sync.dma_start(xi_sb[:], xind[:, :])
        egath = initp.tile([P, NBT * A, DOUT], f32, tag="egath")
        for j in range(NBT * A):
            nc.gpsimd.indirect_dma_start(
                out=egath[:, j, :], out_offset=None,
                in_=iemb[:, :],
                in_offset=bass.IndirectOffsetOnAxis(ap=xi_sb[:, j:j + 1], axis=0))
        eT = const.tile([P, 3, BC], f32)
        for t in range(NBT):
            for kk in range(3):
                pst = psumt.tile([P, P], f32, tag="tr")
                nc.tensor.transpose(
                    pst[:], egath[:, t * A + kk * 2: t * A + kk * 2 + 2, :],
                    ident[:])
                nc.scalar.copy(eT[:, kk, t * P:(t + 1) * P], pst[:, :])

        h_hi, h_lo = [], []
        for m in range(NKH):
            ps = psumg.tile([P, BC], f32, tag="gps")
            for k in range(3):
                nc.tensor.matmul(ps[:], wi_sb[:, k, m * P:(m + 1) * P],
                                 eT[:, k, :], start=(k == 0), stop=(k == 2))
            h = misc.tile([P, BC], f32, tag="hn")
            nc.scalar.activation(h[:], ps[:], AF.Identity, bias=bi_sb[:, m:m + 1])
            hh = state.tile([P, BC], f16, tag=f"hh{m}")
            hl = state.tile([P, BC], f16, tag=f"hl{m}")
            split16(h[:], hh[:], hl[:])
            h_hi.append(hh)
            h_lo.append(hl)

        cT = []
        for j in range(NKH):
            c = state.tile([P, BC], f32, tag=f"cT{j}")
            nc.vector.memset(c[:], 0.0)
            cT.append(c)

        z64 = initp.tile([DOUT, BC], f32, tag="z64")
        nc.vector.memset(z64[:], 0.0)
        embT = state.tile([DOUT, BC], f32, tag="embT")
        nc.scalar.activation(embT[:], z64[:], AF.Identity, bias=sos_sb[:, 0:1])
        embcat = state.tile([P, BC], f16, tag="embcat")
        split16(embT[:], embcat[0:DOUT, :], embcat[DOUT:P, :])

        # ---- recurrence ----
        for t in range(steps):
            # gates^T = W_hh h^T + W_ih emb^T + b  (fp16 x3)  -> i,f,g,o
            # j-interleaved M order: after the 4 gate tiles of cell-slice j
            # are emitted, the cell update for j runs while PE continues
            # with slice j+1 (hides the ACT/DVE tail under matmuls).
            cT_new, hhi_new, hlo_new = [], [], []
            for j in range(NKH):
                gj = {}
                for m in (j, NKH + j, 2 * NKH + j, 3 * NKH + j):
                    ms = slice(m * P, (m + 1) * P)
                    ps = psumg.tile([P, BC], f32, tag="gps")
                    nc.tensor.matmul(ps[:], whhh_sb[:, 0, ms], h_hi[0][:],
                                     start=True, stop=False)
                    for k in range(1, NKH):
                        nc.tensor.matmul(ps[:], whhh_sb[:, k, ms], h_hi[k][:],
                                         start=False, stop=False)
                    for k in range(NKH):
                        nc.tensor.matmul(ps[:], whhl_sb[:, k, ms], h_hi[k][:],
                                         start=False, stop=False)
                    for k in range(NKH):
                        nc.tensor.matmul(ps[:], whhh_sb[:, k, ms], h_lo[k][:],
                                         start=False, stop=False)
                    nc.tensor.matmul(ps[:], wihh_sb[:, ms], embcat[0:DOUT, :],
                                     start=False, stop=False)
                    nc.tensor.matmul(ps[:], wiha_sb[:, ms], embcat[:],
                                     start=False, stop=True)
                    g = gwork.tile([P, BC], f32, tag=f"g{m}")
                    func = AF.Tanh if m // NKH == 2 else AF.Sigmoid
                    nc.scalar.activation(g[:], ps[:], func, bias=bg_sb[:, m:m + 1])
                    gj[m] = g
                # cell slice j: c = f*c + i*g ; h = o * tanh(c) ; split h
                ig = misc.tile([P, BC], f32, tag="ig")
                nc.vector.tensor_mul(ig[:], gj[j][:], gj[2 * NKH + j][:])
                fc = misc.tile([P, BC], f32, tag="fc")
                nc.vector.tensor_mul(fc[:], gj[NKH + j][:], cT[j][:])
                cn = state.tile([P, BC], f32, tag=f"cT{j}")
                nc.vector.tensor_add(cn[:], ig[:], fc[:])
                th = misc.tile([P, BC], f32, tag="th")
                nc.scalar.activation(th[:], cn[:], AF.Tanh)
                hn = misc.tile([P, BC], f32, tag="hn")
                nc.vector.tensor_mul(hn[:], gj[3 * NKH + j][:], th[:])
                hh = state.tile([P, BC], f16, tag=f"hh{j}")
                hl = state.tile([P, BC], f16, tag=f"hl{j}")
                split16(hn[:], hh[:], hl[:])
                cT_new.append(cn)
                hhi_new.append(hh)
                hlo_new.append(hl)
            cT, h_hi, h_lo = cT_new, hhi_new, hlo_new

            # logits = h Wo + bo ; argmax -> sym
            for bt in range(NBT):
                bs = slice(bt * P, (bt + 1) * P)
                lsb = lwork.tile([P, V], f32, tag="lsb")
                for vb in range(2):
                    vs = slice(vb * 512, (vb + 1) * 512)
                    pl = psuml.tile([P, 512], f32, tag="lps")
                    for k in range(NKH):
                        nc.tensor.matmul(pl[:], h_hi[k][:, bs], woh_sb[:, k, vs],
                                         start=(k == 0), stop=False)
                        nc.tensor.matmul(pl[:], h_hi[k][:, bs], wol_sb[:, k, vs],
                                         start=False, stop=False)
                        nc.tensor.matmul(pl[:], h_lo[k][:, bs], woh_sb[:, k, vs],
                                         start=False, stop=(k == NKH - 1))
                    nc.vector.tensor_add(lsb[:, vs], pl[:], bo_sb[:, vs])
                nc.sync.dma_start(
                    log_out[bt * P:(bt + 1) * P, t, :].unsqueeze(1),
                    lsb[:].unsqueeze(1))
                mx8 = misc.tile([P, 8], f32, tag="mx8")
                nc.vector.max(mx8[:], lsb[:])
                mi8 = misc.tile([P, 8], u16, tag="mi8")
                nc.vector.max_index(mi8[:], mx8[:], lsb[:])
                nc.vector.tensor_copy(seqi[:, bt, t:t + 1], mi8[:, 0:1])

            if t == steps - 1:
                break

            # emb = output_emb[sym]: indirect row gather + PE transpose + split
            egn = misc.tile([P, NBT, DOUT], f32, tag="egn")
            for bt in range(NBT):
                nc.gpsimd.indirect_dma_start(
                    out=egn[:, bt, :], out_offset=None,
                    in_=oemb[:, :],
                    in_offset=bass.IndirectOffsetOnAxis(
                        ap=seqi[:, bt, t:t + 1], axis=0))
            embT = state.tile([DOUT, BC], f32, tag="embT")
            for bt in range(NBT):
                pse = psumt.tile([P, P], f32, tag="tr")
                nc.tensor.transpose(pse[:DOUT, :], egn[:, bt, :], ident[:])
                nc.scalar.copy(embT[:, bt * P:(bt + 1) * P], pse[:DOUT, :])
            embcat = state.tile([P, BC], f16, tag="embcat")
            split16(embT[:], embcat[0:DOUT, :], embcat[DOUT:P, :])

        # ---- outputs ----
        for bt in range(NBT):
            nc.sync.dma_start(seq_out[bt * P:(bt + 1) * P, :],
                              seqi[:, bt, :steps])

    nc.finalize()
    return nc


def _split16_np(x):
    hi = x.astype(np.float16)
    lo = (x - hi.astype(np.float32)).astype(np.float16)
    return hi, lo


def _prep_inputs(x, input_emb, output_emb, Wi, bi, W_ih, W_hh, b_ih, b_hh,
                 Wo, bo, sos):
    f = np.float32
    x = np.asarray(x)
    W_hh = np.asarray(W_hh, f)
    W_ih = np.asarray(W_ih, f)
    Wo_ = np.asarray(Wo, f)
    Wi_ = np.asarray(Wi, f)

    whhT = np.ascontiguousarray(W_hh.T.reshape(NKH, P, G4).transpose(1, 0, 2))
    whh_hi, whh_lo = _split16_np(whhT)
    wo_t = np.ascontiguousarray(Wo_.reshape(NKH, P, V).transpose(1, 0, 2))
    wo_hi, wo_lo = _split16_np(wo_t)
    wihT = np.ascontiguousarray(W_ih.T)                    # [64, 2048]
    wih_hi, wih_lo = _split16_np(wihT)

    common = {
        "whh_hi": whh_hi,
        "whh_lo": whh_lo,
        "wih_hi": wih_hi,
        "wih_aux": np.ascontiguousarray(np.concatenate([wih_lo, wih_hi], axis=0)),
        "wo_hi": wo_hi,
        "wo_lo": wo_lo,
        "wi": np.ascontiguousarray(Wi_.reshape(3, P, H).transpose(1, 0, 2)),
        "bias_g": np.ascontiguousarray(
            (np.asarray(b_ih, f) + np.asarray(b_hh, f)).reshape(NM, P).T),
        "bi_col": np.ascontiguousarray(np.asarray(bi, f).reshape(NKH, P).T),
        "bo_bc": np.ascontiguousarray(
            np.broadcast_to(np.asarray(bo, f), (P, V))),
        "sos_col": np.ascontiguousarray(np.asarray(sos, f).reshape(DOUT, 1)),
        "oemb": np.ascontiguousarray(np.asarray(output_emb, f)),
        "iemb": np.ascontiguousarray(np.asarray(input_emb, f)),
    }
    in_maps = []
    for c in range(NCORES):
        xc = x[c * BC:(c + 1) * BC].astype(np.int64)
        # xind[p, bt*A + a] = x[bt*128 + p, a]
        jj = np.arange(NBT * A)
        pp = np.arange(P)
        mat = xc[(jj[None, :] // A) * P + pp[:, None], jj[None, :] % A]
        m = dict(common)
        m["xind"] = np.ascontiguousarray(mat.astype(np.int32))
        in_maps.append(m)
    return in_maps


def _get_nc(steps=L):
    if steps not in _CACHE:
        _CACHE[steps] = _build_nc(steps)
    return _CACHE[steps]


def kernel(x, input_emb, output_emb, Wi, bi, W_ih, W_hh, b_ih, b_hh, Wo, bo,
           sos, _steps=L, _trace=False):
    from concourse.bass_utils import run_bass_kernel_spmd

    nc = _get_nc(_steps)
    in_maps = _prep_inputs(x, input_emb, output_emb, Wi, bi, W_ih, W_hh,
                           b_ih, b_hh, Wo, bo, sos)
    res = run_bass_kernel_spmd(nc, in_maps, list(range(NCORES)), trace=_trace)
    kernel.last_result = res
    seq = np.concatenate([res.results[c]["seq"] for c in range(NCORES)], axis=0)
    logits = np.concatenate([res.results[c]["logits"] for c in range(NCORES)],
                            axis=0)
    if np.asarray(x).dtype == np.int64:
        seq = seq.astype(np.int64)
    else:
        seq = seq.astype(np.int32)
    return seq, logits
